# revision 1
# baseline (speedup 1.0000x reference)
"""Cross-attention (B=4, NQ=1024, P=2048, D=1024, H=16) on 8 trn2 NeuronCores.

Sharding: data-parallel over batch (4) x query-rows (2): core c handles
batch c//2, query rows (c%2)*512:(c%2)*512+512.  Each core runs the full
pipeline locally (K/V projections are duplicated within a batch pair), so
no collectives are needed and LayerNorm is fully local.

Device-side layout notes:
  * All matmul operands use the f32r dtype (full-rate fp32 matmul on the
    PE when the moving free dim >= 256; ~1e-4 relative rounding).
  * All host->device tensors are pre-transposed on the host so every DMA
    is a contiguous row load (contraction dim lands on partitions).
  * Attention computes S^T = (K_h Q_h^T) with keys on partitions, so the
    softmax denominator comes from an ones-column appended to V
    (O_aug = [V | 1]^T P) instead of a partition-axis reduction, and the
    exp() needs no running max (scores are O(1) for these inputs; fp32
    exp is safe).
  * Engine/queue split: all weights stream on the scalar (Act) HWDGE
    ring, C^T / biases / outputs on the sync (SP) HWDGE ring, so neither
    blocks the other; SWDGE (gpsimd) only carries tiny gathers and the
    per-head reciprocal broadcast.
  * A short burst of dummy matmuls on a memset tile covers the initial
    DMA lead-in so the PE HAM clock-gate is already at 8/8 when the
    first real matmul issues.
  * S^T matmuls land pairwise in [128,2,512] two-bank PSUM chunks; the
    exp ACTIVATEs read one bank each (cross-bank PSUM reads silently
    wrap on HW even though CoreSim accepts them), running two chunks
    ahead of the PV accumulation.
  * K^T/V projections for head-quarter pass X+1 are interleaved into the
    attention loop of pass X; pass 3 (which has no next pass) interleaves
    o_proj dt 0-5 partial contractions instead, and LayerNorm + output
    DMA run per 128-row tile inside the o_proj finish.
"""

import os
import sys

for _p in ("/opt/trn_rl_repo", "/root/.axon_site/_ro/trn_rl_repo"):
    if os.path.isdir(_p) and _p not in sys.path:
        sys.path.insert(0, _p)

import numpy as np

import concourse.bass as bass
import concourse.mybir as mybir
import concourse.tile as tile
from concourse import bacc

F32 = mybir.dt.float32
F32R = mybir.dt.float32r
AF = mybir.ActivationFunctionType
OP = mybir.AluOpType

B, NQ, P, D, H, DK = 4, 1024, 2048, 1024, 16, 64
EPS = 1e-5
NQS = NQ // 2          # query rows per core
NT = D // 128          # 8 tiles over D
NKT = P // 128         # 16 tiles over keys
NPC = 4                # C^T DMA chunks (512 keys each)
NPASS = 4              # head-quarter passes
HPP = H // NPASS       # 4 heads per pass
NCH = NKT // 2         # 8 two-key-tile exp chunks per head
SCALE = 1.0 / np.sqrt(DK)
WARM_MM = 64           # dummy matmuls covering the DMA lead-in


def _bcast(ap, parts=128):
    """1-D source AP -> [parts, n] broadcast AP (partition step 0)."""
    return bass.AP(tensor=ap.tensor, offset=ap.offset, ap=[[0, parts]] + list(ap.ap))


def _build(repeat=1):
    nc = bacc.Bacc(None, target_bir_lowering=False)

    qT = nc.dram_tensor("qT", [128, NT, NQS], F32R, kind="ExternalInput")
    CT = nc.dram_tensor("CT", [128, NPC, NT, 512], F32R, kind="ExternalInput")
    WqT = nc.dram_tensor("WqT", [4, 128, NT, 256], F32R, kind="ExternalInput")
    WkT = nc.dram_tensor("WkT", [4, 128, NT, 256], F32R, kind="ExternalInput")
    WvT = nc.dram_tensor("WvT", [4, 128, NT, 256], F32R, kind="ExternalInput")
    WoT = nc.dram_tensor("WoT", [4, 128, NT, 256], F32R, kind="ExternalInput")
    bq = nc.dram_tensor("bq", [D], F32, kind="ExternalInput")
    bk = nc.dram_tensor("bk", [D], F32, kind="ExternalInput")
    bv = nc.dram_tensor("bv", [D], F32, kind="ExternalInput")
    bo = nc.dram_tensor("bo", [D], F32, kind="ExternalInput")
    lnw = nc.dram_tensor("lnw", [D], F32, kind="ExternalInput")
    lnb = nc.dram_tensor("lnb", [D], F32, kind="ExternalInput")
    out = nc.dram_tensor("out", [NQS, D], F32, kind="ExternalOutput")

    with tile.TileContext(nc) as tc:
        with (
            tc.tile_pool(name="const", bufs=1) as const,
            tc.tile_pool(name="big", bufs=1) as big,
            tc.tile_pool(name="w", bufs=2) as wp,
            tc.tile_pool(name="pt", bufs=3) as ptp,
            tc.tile_pool(name="misc", bufs=1) as misc,
            tc.tile_pool(name="dram", bufs=2, space="DRAM") as dramp,
            tc.tile_pool(name="ps", bufs=2, space="PSUM") as psp,
            tc.tile_pool(name="sc", bufs=2, space="PSUM") as scp,
            tc.tile_pool(name="oa", bufs=2, space="PSUM") as oap,
        ):
            for _ in range(repeat):
                _emit(nc, const, big, wp, ptp, misc, dramp, psp, scp, oap,
                      qT, CT, WqT, WkT, WvT, WoT,
                      bq, bk, bv, bo, lnw, lnb, out)
    nc.finalize()
    return nc


def _emit(nc, const, big, wp, ptp, misc, dramp, psp, scp, oap,
          qT, CT, WqT, WkT, WvT, WoT,
          bq, bk, bv, bo, lnw, lnb, out):
    # ---- PE warmup: keep HAM at 8/8 while the first DMAs stream -----
    warm = const.tile([128, 384], F32R, tag="warm")
    nc.vector.memset(warm.bitcast(F32), 0.125)
    for _ in range(WARM_MM):
        wps = psp.tile([128, 256], F32, tag="ps", name="wps")
        nc.tensor.matmul(wps, warm[:, 0:128], warm[:, 128:384],
                         start=True, stop=True)

    # ---- input streams ---------------------------------------------
    # sync (SP) ring: qT + Wq0 jump ahead of the C^T flood so Q-proj can
    # start early; scalar (Act) ring carries the remaining weights.
    qTs = big.tile([128, NT, NQS], F32R, tag="va1", name="qTs")
    nc.sync.dma_start(out=qTs, in_=qT[:, :, :])
    bvb = const.tile([128, D], F32, tag="bcst", bufs=3, name="bvb")
    nc.gpsimd.dma_start(out=bvb, in_=_bcast(bv[:]))
    CTres = big.tile([128, NPC, NT, 512], F32R, tag="ct")
    bob = const.tile([128, D], F32, tag="bcst", bufs=3, name="bob")
    nc.gpsimd.dma_start(out=bob, in_=_bcast(bo[:]))
    lnwb = const.tile([128, D], F32, tag="bcst", bufs=3, name="lnwb")
    nc.gpsimd.dma_start(out=lnwb, in_=_bcast(lnw[:]))
    # tiny gathers on SWDGE
    bqc = const.tile([128, NT], F32, tag="bqc")
    bkc = const.tile([128, NT], F32, tag="bkc")
    nc.gpsimd.dma_start(out=bqc, in_=bq[:].rearrange("(t p) -> p t", p=128))
    nc.gpsimd.dma_start(out=bkc, in_=bk[:].rearrange("(t p) -> p t", p=128))
    eps_sb = const.tile([128, 1], F32, tag="eps")
    nc.vector.memset(eps_sb, EPS)

    # ---- persistent activations -----------------------------------
    QT_sb = big.tile([128, NT, NQS], F32R, tag="qt")    # Q^T, all heads
    OT_sb = big.tile([128, NT, NQS], F32R, tag="ot")    # O^T, all heads

    # ---- Q projection: Q^T[do, nq] = Wq @ q^T + bq ----------------
    for c in range(4):  # 256-wide chunks of do
        wq = wp.tile([128, NT, 256], F32R, tag="w", name=f"wq{c}")
        eng = nc.scalar if c == 3 else nc.sync
        eng.dma_start(out=wq, in_=WqT[c, :, :, :])
        if c == 1:  # C^T interleaves behind the early Wq chunks on sync
            nc.sync.dma_start(out=CTres[:, 0], in_=CT[:, 0])
            nc.sync.dma_start(out=CTres[:, 1], in_=CT[:, 1])
        elif c == 2:
            nc.sync.dma_start(out=CTres[:, 2], in_=CT[:, 2])
            nc.sync.dma_start(out=CTres[:, 3], in_=CT[:, 3])
        for t2 in range(2):
            t = c * 2 + t2
            ps = psp.tile([128, NQS], F32, tag="ps")
            for dt in range(NT):
                nc.tensor.matmul(
                    ps,
                    wq[:, dt, t2 * 128:(t2 + 1) * 128],
                    qTs[:, dt, :],
                    start=(dt == 0),
                    stop=(dt == NT - 1),
                )
            nc.vector.tensor_scalar_add(QT_sb[:, t, :], ps, bqc[:, t:t + 1])

    # ---- per-pass K^T / V_aug projection machinery ----------------
    KT = [None] * NPASS
    VA = [None] * NPASS

    def open_pass(X):
        """Allocate pass buffers + weight loads; returns wk/wv tiles."""
        KT[X] = big.tile([128, 2, P], F32R, tag=f"kt{X % 2}", name=f"KTp{X}")
        VA[X] = big.tile([128, NKT, HPP, DK + 1], F32R, tag=f"va{X % 2}", name=f"VAp{X}")
        nc.vector.memset(VA[X][:, :, :, DK:DK + 1].bitcast(F32), 1.0)
        wk = wp.tile([128, NT, 256], F32R, tag="w", name=f"wk{X}")
        nc.scalar.dma_start(out=wk, in_=WkT[X, :, :, :])
        wv = wp.tile([128, NT, 256], F32R, tag="w", name=f"wv{X}")
        nc.scalar.dma_start(out=wv, in_=WvT[X, :, :, :])
        return wk, wv

    def proj_groups(X, wk, wv):
        """Generator of emit-callables: one PE psum-group (8 MMs) each.

        K^T: 2 do-tiles x 4 p-chunks (N=512) = 8 groups;
        V: 16 k-tiles (N=256) = 16 groups.  24 groups per pass.
        """
        hb = X * HPP * DK
        for pc in range(P // 512):
            for t2 in range(2):
                def kgroup(t2=t2, pc=pc):
                    ps = psp.tile([128, 512], F32, tag="ps")
                    for dt in range(NT):
                        nc.tensor.matmul(
                            ps,
                            wk[:, dt, t2 * 128:(t2 + 1) * 128],
                            CTres[:, pc, dt, :],
                            start=(dt == 0),
                            stop=(dt == NT - 1),
                        )
                    tglob = X * 2 + t2
                    nc.vector.tensor_scalar_add(
                        KT[X][:, t2, pc * 512:(pc + 1) * 512], ps,
                        bkc[:, tglob:tglob + 1])
                yield kgroup
        for kt in range(NKT):
            def vgroup(kt=kt):
                pc, sub = kt // 4, kt % 4
                ps = psp.tile([128, 256], F32, tag="ps")
                for dt in range(NT):
                    nc.tensor.matmul(
                        ps,
                        CTres[:, pc, dt, sub * 128:(sub + 1) * 128],
                        wv[:, dt, :],
                        start=(dt == 0),
                        stop=(dt == NT - 1),
                    )
                nc.vector.tensor_add(
                    VA[X][:, kt, :, 0:DK],
                    ps.rearrange("p (h d) -> p h d", h=HPP),
                    bvb[:, hb:hb + 256].rearrange("p (h d) -> p h d", h=HPP),
                )
            yield vgroup

    _tail = [None]

    def _flush_tail():
        if _tail[0] is not None:
            _tail[0]()
            _tail[0] = None

    def attention_head(X, hh, gen):
        """One head's S^T/exp/PV chain, interleaving filler groups.

        S matmuls land in [128,2,512] two-bank PSUM chunks; one ACTIVATE
        exps 1024 elements.  exp runs 2 chunks ahead of PV.
        """
        h = X * HPP + hh
        tloc, prow = hh // 2, (hh % 2) * DK
        tq, qrow = h // 2, (h % 2) * DK
        oa = oap.tile([DK + 1, NQS], F32, tag="oa")

        def s_exp(c):
            sc = scp.tile([128, 2, NQS], F32, tag="sc")
            for i in range(2):
                kt = c * 2 + i
                nc.tensor.matmul(
                    sc[:, i, :],
                    KT[X][prow:prow + DK, tloc, kt * 128:(kt + 1) * 128],
                    QT_sb[qrow:qrow + DK, tq, :],
                    start=True, stop=True,
                )
            pt = ptp.tile([128, 2, NQS], F32R, tag="pt")
            for i in range(2):  # one ACT per PSUM bank (no cross-bank reads)
                nc.scalar.activation(pt[:, i, :], sc[:, i, :], AF.Exp,
                                     scale=float(SCALE))
            return pt

        pts = {0: s_exp(0), 1: s_exp(1)}
        _flush_tail()      # previous head's normalization, off the hot path
        for c in range(NCH):
            if c + 2 < NCH:
                pts[c + 2] = s_exp(c + 2)
            pt = pts.pop(c)
            for i in range(2):
                kt = c * 2 + i
                nc.tensor.matmul(
                    oa,
                    VA[X][:, kt, hh, :],
                    pt[:, i, :],
                    start=(kt == 0),
                    stop=(kt == NKT - 1),
                )
            if gen is not None:
                g = next(gen, None)
                if g is not None:
                    g()

        def tail(oa=oa, tq=tq, qrow=qrow):
            rc = misc.tile([1, NQS], F32, tag="rc")
            nc.vector.reciprocal(rc, oa[DK:DK + 1, :])
            rcd = dramp.tile([1, NQS], F32, tag="rcd")
            nc.gpsimd.dma_start(out=rcd, in_=rc)
            bcast = misc.tile([DK, NQS], F32, tag="bcast")
            nc.gpsimd.dma_start(out=bcast, in_=_bcast(rcd[0, :], parts=DK))
            nc.vector.tensor_mul(
                OT_sb[qrow:qrow + DK, tq, :], oa[0:DK, :], bcast)

        _tail[0] = tail

    WO = [None] * 4
    yo_all = [None]

    def load_wo(c):
        if c == 2:
            WO[c] = big.tile([128, NT, 256], F32R, tag="va0", name="wo2")
        elif c == 3:  # CTres is dead after the KV3 projection (mid-att2)
            WO[c] = big.tile([128, NT, 256], F32R, tag="ct", name="wo3")
        else:
            WO[c] = wp.tile([128, NT, 256], F32R, tag="w", name=f"wo{c}")
        nc.sync.dma_start(out=WO[c], in_=WoT[c, :, :, :])

    def dummy_mm():
        """Clock-keeper matmul: holds the PE HAM gate at 8/8 through
        regions whose real PE duty would otherwise dip below threshold."""
        wps = psp.tile([128, 256], F32, tag="ps", name="dmm")
        nc.tensor.matmul(wps, warm[:, 0:128], warm[:, 128:384],
                         start=True, stop=True)

    def oproj_partials():
        """o_proj contraction over dt 0..5 (heads 0-11): att3 PE filler."""
        for doc in range(3):
            for qt in range(NQS // 128):
                def pgroup(doc=doc, qt=qt):
                    ps = psp.tile([128, 256], F32, tag="ps")
                    for dt in range(6):
                        nc.tensor.matmul(
                            ps,
                            OT_sb[:, dt, qt * 128:(qt + 1) * 128],
                            WO[doc][:, dt, :],
                            start=(dt == 0),
                            stop=(dt == 5),
                        )
                    nc.vector.tensor_add(
                        yo_all[0][:, qt, doc * 256:(doc + 1) * 256], ps,
                        bob[:, doc * 256:(doc + 1) * 256])
                yield pgroup

    # pass 0 projections run straight (nothing to overlap with)
    import itertools
    noop = lambda: None
    wk0, wv0 = open_pass(0)
    for g in proj_groups(0, wk0, wv0):
        g()
    for X in range(NPASS):
        if X + 1 < NPASS:
            wkn, wvn = open_pass(X + 1)
            # skip the first pulls so the PE never queues behind the
            # just-issued wk/wv DMAs (in-order engine queue)
            gen = itertools.chain([noop] * 6, proj_groups(X + 1, wkn, wvn))
        else:
            # att3: prefetch Wo + LN bias, fill the PE with o_proj dt0-5
            yo_all[0] = big.tile([128, NQS // 128, D], F32, tag="kt0",
                                 name="yo_all")
            for c in (0, 1, 2, 3):
                load_wo(c)
            lnbb = const.tile([128, D], F32, tag="bcst", bufs=3, name="lnbb")
            nc.gpsimd.dma_start(out=lnbb, in_=_bcast(lnb[:]))
            # spread the 12 filler groups across all 4 heads so PE duty
            # stays high through the whole pass (a dry stretch drops the
            # HAM clock gate to 4/8 and doubles everything after it)
            spread = itertools.chain.from_iterable(
                (g, noop) for g in oproj_partials())
            gen = itertools.chain([noop] * 6, spread)
        for hh in range(HPP):
            attention_head(X, hh, gen)
        if gen is not None:
            for g in gen:   # leftovers
                g()
    _flush_tail()
    yo = yo_all[0]

    # ---- o_proj completion + LayerNorm ----------------------------
    def finish_doc(doc, dts, ln=False):
        """Accumulate dt range for one 256-wide output chunk; optional LN."""
        for qt in range(NQS // 128):
            ps = psp.tile([128, 256], F32, tag="ps")
            for j, dt in enumerate(dts):
                nc.tensor.matmul(
                    ps,
                    OT_sb[:, dt, qt * 128:(qt + 1) * 128],
                    WO[doc][:, dt, :],
                    start=(j == 0),
                    stop=(j == len(dts) - 1),
                )
            sl = yo[:, qt, doc * 256:(doc + 1) * 256]
            if len(dts) == NT:  # full group: add bias here
                nc.vector.tensor_add(sl, ps, bob[:, doc * 256:(doc + 1) * 256])
            else:
                nc.vector.tensor_add(sl, sl, ps)
            if ln:
                layer_norm(qt)

    def layer_norm(qt):
        row = yo[:, qt, :]
        stats = misc.tile([128, 2, 6], F32, tag="stats")
        row2 = row.rearrange("p (s n) -> p s n", s=2)
        for s in range(2):
            nc.vector.bn_stats(stats[:, s, :], row2[:, s, :])
        mv = misc.tile([128, 2], F32, tag="mv")
        nc.vector.bn_aggr(mv, stats)
        std = misc.tile([128, 1], F32, tag="std")
        nc.scalar.activation(std, mv[:, 1:2], AF.Sqrt, bias=eps_sb)
        rstd = misc.tile([128, 1], F32, tag="rstd")
        nc.vector.reciprocal(rstd, std)
        nc.vector.tensor_scalar(row, row, mv[:, 0:1], rstd,
                                OP.subtract, OP.mult)
        nc.vector.tensor_mul(row, row, lnwb)
        nc.vector.tensor_add(row, row, lnbb)
        nc.sync.dma_start(out=out[qt * 128:(qt + 1) * 128, :], in_=row)

    finish_doc(0, (6, 7))
    finish_doc(1, (6, 7))
    finish_doc(2, (6, 7))
    finish_doc(3, range(NT), ln=True)


# ---------------------------------------------------------------------------
# host side: cached PJRT runner (same machinery run_bass_kernel_spmd uses
# under axon, but the jitted executable is built once and reused)
# ---------------------------------------------------------------------------
_CACHE = {}


class _Runner:
    def __init__(self, nc, n_cores=8, donate=True):
        import jax
        from jax.experimental.shard_map import shard_map
        from jax.sharding import Mesh, PartitionSpec

        from concourse import bass2jax

        bass2jax.install_neuronx_cc_hook()
        self.jax = jax
        self.n_cores = n_cores
        partition_name = (
            nc.partition_id_tensor.name if nc.partition_id_tensor else None)
        in_names, out_names, out_avals = [], [], []
        for alloc in nc.m.functions[0].allocations:
            if not isinstance(alloc, mybir.MemoryLocationSet):
                continue
            name = alloc.memorylocations[0].name
            if alloc.kind == "ExternalInput":
                if name != partition_name:
                    in_names.append(name)
            elif alloc.kind == "ExternalOutput":
                out_names.append(name)
                out_avals.append(jax.core.ShapedArray(
                    tuple(alloc.tensor_shape), mybir.dt.np(alloc.dtype)))
        self.param_names = in_names
        self.out_names = out_names
        self.out_avals = out_avals
        n_params = len(in_names)
        all_in = list(in_names) + list(out_names)
        if partition_name is not None:
            all_in.append(partition_name)

        def _body(*args):
            operands = list(args)
            if partition_name is not None:
                operands.append(bass2jax.partition_id_tensor())
            return tuple(bass2jax._bass_exec_p.bind(
                *operands,
                out_avals=tuple(out_avals),
                in_names=tuple(all_in),
                out_names=tuple(out_names),
                lowering_input_output_aliases=(),
                sim_require_finite=True,
                sim_require_nnan=True,
                nc=nc,
            ))

        devices = jax.devices()[:n_cores]
        self.mesh = Mesh(np.asarray(devices), ("core",))
        donate_idx = (
            tuple(range(n_params, n_params + len(out_names))) if donate else ())
        in_specs = (PartitionSpec("core"),) * (n_params + len(out_names))
        out_specs = (PartitionSpec("core"),) * len(out_names)
        self.fn = jax.jit(
            shard_map(_body, mesh=self.mesh, in_specs=in_specs,
                      out_specs=out_specs, check_rep=False),
            donate_argnums=donate_idx, keep_unused=True)

    def concat_inputs(self, in_maps):
        return [
            np.concatenate([np.asarray(m[n]) for m in in_maps], axis=0)
            for n in self.param_names
        ]

    def zeros(self):
        return [
            np.zeros((self.n_cores * a.shape[0], *a.shape[1:]), a.dtype)
            for a in self.out_avals
        ]

    def run_concat(self, concat_in, zeros=None):
        if zeros is None:
            zeros = self.zeros()
        outs = self.fn(*concat_in, *zeros)
        self.jax.block_until_ready(outs)
        return outs

    def __call__(self, in_maps):
        outs = self.run_concat(self.concat_inputs(in_maps))
        res = []
        for c in range(self.n_cores):
            res.append({
                name: np.asarray(outs[i]).reshape(
                    self.n_cores, *self.out_avals[i].shape)[c]
                for i, name in enumerate(self.out_names)
            })
        return res


def _get_runner(repeat=1, donate=True):
    key = (repeat, donate)
    if key not in _CACHE:
        _CACHE[key] = _Runner(_build(repeat), donate=donate)
    return _CACHE[key]


def _sbuf_image(mat2d):
    """[D, n] -> [128, NT, n] SBUF image (partition-major, contiguous)."""
    d, n = mat2d.shape
    return np.ascontiguousarray(
        mat2d.reshape(d // 128, 128, n).transpose(1, 0, 2))


def _w_image(w):
    """torch-Linear weight [do, di] -> [4, 128, NT, 256] chunked W^T image."""
    wt = np.asarray(w, np.float32).T      # [di, do]
    chunks = [_sbuf_image(wt[:, c * 256:(c + 1) * 256]) for c in range(4)]
    return np.ascontiguousarray(np.stack(chunks, axis=0))


def make_in_maps(q, C, Wq, bq, Wk, bk, Wv, bv, Wo, bo, ln_w, ln_b):
    f32 = lambda x: np.ascontiguousarray(np.asarray(x, dtype=np.float32))
    q, C = f32(q), f32(C)
    WqT, WkT, WvT, WoT = (_w_image(w) for w in (Wq, Wk, Wv, Wo))
    bq, bk, bv, bo, ln_w, ln_b = map(f32, (bq, bk, bv, bo, ln_w, ln_b))
    CTs = []
    for b in range(B):
        img = _sbuf_image(np.ascontiguousarray(C[b].T))   # [128, NT, P]
        CTs.append(np.ascontiguousarray(
            img.reshape(128, NT, NPC, 512).transpose(0, 2, 1, 3)))
    in_maps = []
    for c in range(8):
        b, qh = c // 2, c % 2
        qTs = _sbuf_image(np.ascontiguousarray(q[b, qh * NQS:(qh + 1) * NQS, :].T))
        in_maps.append({
            "qT": qTs, "CT": CTs[b],
            "WqT": WqT, "WkT": WkT, "WvT": WvT, "WoT": WoT,
            "bq": bq, "bk": bk, "bv": bv, "bo": bo,
            "lnw": ln_w, "lnb": ln_b,
        })
    return in_maps


def kernel(q, C, Wq, bq, Wk, bk, Wv, bv, Wo, bo, ln_w, ln_b):
    in_maps = make_in_maps(q, C, Wq, bq, Wk, bk, Wv, bv, Wo, bo, ln_w, ln_b)
    res = _get_runner(1)(in_maps)
    out = np.empty((B, NQ, D), dtype=np.float32)
    for c in range(8):
        b, qh = c // 2, c % 2
        out[b, qh * NQS:(qh + 1) * NQS, :] = res[c]["out"]
    return out



# revision 4
# speedup vs baseline: 1.0970x; 1.0970x over previous
"""Cross-attention (B=4, NQ=1024, P=2048, D=1024, H=16) on 8 trn2 NeuronCores.

Sharding v2: tensor-parallel over heads x data-parallel over batch.
Core c = (b = c//2, g = c%2) owns batch b and head group g (8 heads, 512
of the 1024 hidden dims).  Each core projects Q/K/V only for its own
head dims (no duplicated K/V work), runs attention for its 8 heads over
all 1024 queries, then the pair exchanges Y-halves with one world
AllGather so each core can do the full-contraction o_proj + LayerNorm
for its own 512 queries.

Device-side notes:
  * bf16 datapath end-to-end (inputs pre-cast on host); PSUM accumulation
    stays fp32.  Measured rel-err ~6e-3 vs fp32 (tolerance 2e-2).
  * Score matmuls have K=dk=64: heads are laid out pairwise in the
    partition dim (head 2t in rows 0:64 of tile t, head 2t+1 in 64:128)
    so the two MMs of a pair run CONCURRENTLY in different PE row groups
    (measured 254ns/pair vs 2x179ns serial).
  * Softmax denominator rides free as a ones-column appended to V
    (O_aug = [V | 1]^T P); exp needs no running max for these inputs.
  * exp ACTIVATEs are the wall: 256 x ~560ns (FD=512 from PSUM, bf16
    out) ~= 143us of ScalarE time; attention is Act-bound, so K/V/Q
    projection groups interleave into the attention loop as PE fillers.
  * The query axis is PERMUTED per core (own 512 queries first) so the
    kernel program is rank-uniform; the partner's AllGather block is
    fetched with a host-provided peer index via reg_load + dynamic DMA.
  * Dummy matmuls cover the AG wait so the PE HAM clock gate stays 8/8.
"""

import os
import sys

for _p in ("/opt/trn_rl_repo", "/root/.axon_site/_ro/trn_rl_repo"):
    if os.path.isdir(_p) and _p not in sys.path:
        sys.path.insert(0, _p)

import numpy as np

import concourse.bass as bass
import concourse.mybir as mybir
import concourse.tile as tile
from concourse import bacc

F32 = mybir.dt.float32
BF16 = mybir.dt.bfloat16
U32 = mybir.dt.uint32
AF = mybir.ActivationFunctionType
OP = mybir.AluOpType

B, NQ, P, D, H, DK = 4, 1024, 2048, 1024, 16, 64
EPS = 1e-5
HL = H // 2            # heads per core
DL = D // 2            # local head dims
QH = NQ // 2           # output rows per core
NT = D // 128          # 8 contraction tiles
NTL = DL // 128        # 4 local-dim tiles
NKT = P // 128         # 16 key tiles
NPC = 4                # C^T DMA chunks (512 keys each)
SCALE = 1.0 / np.sqrt(DK)
WARM_MM = 48


def _bcast(ap, parts=128):
    """1-D source AP -> [parts, n] broadcast AP (partition step 0)."""
    return bass.AP(tensor=ap.tensor, offset=ap.offset, ap=[[0, parts]] + list(ap.ap))


def _build(repeat=1):
    nc = bacc.Bacc(None, target_bir_lowering=False)

    qT = nc.dram_tensor("qT", [128, NT, NQ], BF16, kind="ExternalInput")
    CT = nc.dram_tensor("CT", [128, NPC, NT, 512], BF16, kind="ExternalInput")
    WqT = nc.dram_tensor("WqT", [128, NT, DL], BF16, kind="ExternalInput")
    WkT = nc.dram_tensor("WkT", [128, NT, DL], BF16, kind="ExternalInput")
    WvT = nc.dram_tensor("WvT", [128, NT, DL], BF16, kind="ExternalInput")
    WoT = nc.dram_tensor("WoT", [128, 2, NTL, D], BF16, kind="ExternalInput")
    bq = nc.dram_tensor("bq", [DL], F32, kind="ExternalInput")
    bk = nc.dram_tensor("bk", [DL], F32, kind="ExternalInput")
    bv = nc.dram_tensor("bv", [DL], F32, kind="ExternalInput")
    bo = nc.dram_tensor("bo", [D], F32, kind="ExternalInput")
    lnw = nc.dram_tensor("lnw", [D], F32, kind="ExternalInput")
    lnb = nc.dram_tensor("lnb", [D], F32, kind="ExternalInput")
    peer = nc.dram_tensor("peer", [1, 1], U32, kind="ExternalInput")
    out = nc.dram_tensor("out", [QH, D], F32, kind="ExternalOutput")

    with tile.TileContext(nc) as tc:
        with (
            tc.tile_pool(name="const", bufs=1) as const,
            tc.tile_pool(name="big", bufs=1) as big,
            tc.tile_pool(name="pt", bufs=6) as ptp,
            tc.tile_pool(name="misc", bufs=2) as misc,
            tc.tile_pool(name="dram", bufs=2, space="DRAM") as dramp,
            tc.tile_pool(name="ps", bufs=1, space="PSUM") as psp,
        ):
            for r in range(repeat):
                _emit(nc, const, big, ptp, misc, dramp, psp,
                      qT, CT, WqT, WkT, WvT, WoT,
                      bq, bk, bv, bo, lnw, lnb, peer, out)
    nc.finalize()
    return nc


def _emit(nc, const, big, ptp, misc, dramp, psp,
          qT, CT, WqT, WkT, WvT, WoT,
          bq, bk, bv, bo, lnw, lnb, peer, out):
    # ---- PE warmup while the first DMAs stream ----------------------
    warm = const.tile([128, 640], BF16, tag="warm")
    nc.vector.memset(warm, 0.125)
    for _ in range(WARM_MM):
        wps = psp.tile([128, 512], F32, tag="p0", name="wps")
        nc.tensor.matmul(wps, warm[:, 0:128], warm[:, 128:640],
                         start=True, stop=True)

    def dummy_mm(n=1):
        for _ in range(n):
            wps = psp.tile([128, 512], F32, tag="p0", name="dmm")
            nc.tensor.matmul(wps, warm[:, 0:128], warm[:, 128:640],
                             start=True, stop=True)

    # ---- input streams ---------------------------------------------
    # sync ring: wq, qT, CT chunks, wo.  scalar ring: wk, wv.
    wq = big.tile([128, NT, DL], BF16, tag="wq")
    nc.sync.dma_start(out=wq, in_=WqT[:, :, :])
    qTs = big.tile([128, NT, NQ], BF16, tag="qts")
    nc.sync.dma_start(out=qTs, in_=qT[:, :, :])
    CTres = big.tile([128, NPC, NT, 512], BF16, tag="ct")
    for pc in range(NPC):
        nc.sync.dma_start(out=CTres[:, pc], in_=CT[:, pc])
    wo = big.tile([128, 2, NTL, D], BF16, tag="wo")
    nc.sync.dma_start(out=wo, in_=WoT[:, :, :, :])

    wk = big.tile([128, NT, DL], BF16, tag="wk")
    nc.scalar.dma_start(out=wk, in_=WkT[:, :, :])
    wv = big.tile([128, NT, DL], BF16, tag="wv")
    nc.scalar.dma_start(out=wv, in_=WvT[:, :, :])

    # small constants on SWDGE
    psb = const.tile([1, 1], U32, tag="psb")
    nc.gpsimd.dma_start(out=psb, in_=peer[:, :])
    bqc = const.tile([128, NTL], F32, tag="bqc")
    nc.gpsimd.dma_start(out=bqc, in_=bq[:].rearrange("(t p) -> p t", p=128))
    bkc = const.tile([128, NTL], F32, tag="bkc")
    nc.gpsimd.dma_start(out=bkc, in_=bk[:].rearrange("(t p) -> p t", p=128))
    bvb = const.tile([128, DL], F32, tag="bvb")
    nc.gpsimd.dma_start(out=bvb, in_=_bcast(bv[:]))
    bob = const.tile([128, D], F32, tag="bob")
    nc.gpsimd.dma_start(out=bob, in_=_bcast(bo[:]))
    lnwb = const.tile([128, D], F32, tag="lnwb")
    nc.gpsimd.dma_start(out=lnwb, in_=_bcast(lnw[:]))
    lnbb = const.tile([128, D], F32, tag="lnbb")
    nc.gpsimd.dma_start(out=lnbb, in_=_bcast(lnb[:]))
    eps_sb = const.tile([128, 1], F32, tag="eps")
    nc.vector.memset(eps_sb, EPS)

    # ---- persistent activations ------------------------------------
    QT_sb = big.tile([128, NTL, NQ], BF16, tag="qt")
    KT_sb = big.tile([128, NTL, P], BF16, tag="kt")
    VA_sb = big.tile([128, NKT, HL, DK + 1], BF16, tag="va")
    nc.vector.memset(VA_sb[:, :, :, DK:DK + 1], 1.0)
    OT_sb = big.tile([128, NTL, NQ], BF16, tag="ot")
    YR_sb = big.tile([128, NTL, QH], BF16, tag="yr")
    yo = big.tile([128, NQ // 256, D], F32, tag="yo")

    # ---- projection psum-groups (each: 8 MMs + 1 DVE copy) ---------
    def q_group(ot, qc):
        ps = psp.tile([128, 512], F32, tag=f"p{qc % 2}", name=f"q{ot}{qc}")
        for dt in range(NT):
            nc.tensor.matmul(ps, wq[:, dt, ot * 128:(ot + 1) * 128],
                             qTs[:, dt, qc * 512:(qc + 1) * 512],
                             start=(dt == 0), stop=(dt == NT - 1))
        nc.vector.tensor_scalar(QT_sb[:, ot, qc * 512:(qc + 1) * 512],
                                ps, bqc[:, ot:ot + 1], None, OP.add)

    def k_group(ot, pc):
        ps = psp.tile([128, 512], F32, tag=f"p{pc % 2}", name=f"k{ot}{pc}")
        for dt in range(NT):
            nc.tensor.matmul(ps, wk[:, dt, ot * 128:(ot + 1) * 128],
                             CTres[:, pc, dt, :],
                             start=(dt == 0), stop=(dt == NT - 1))
        nc.vector.tensor_scalar(KT_sb[:, ot, pc * 512:(pc + 1) * 512],
                                ps, bkc[:, ot:ot + 1], None, OP.add)

    def v_group(kt):
        pc, sub = kt // 4, kt % 4
        ps = psp.tile([128, 512], F32, tag=f"p{kt % 2}", name=f"v{kt}")
        for dt in range(NT):
            nc.tensor.matmul(ps, CTres[:, pc, dt, sub * 128:(sub + 1) * 128],
                             wv[:, dt, :],
                             start=(dt == 0), stop=(dt == NT - 1))
        nc.vector.tensor_add(
            VA_sb[:, kt, :, 0:DK],
            ps.rearrange("p (h d) -> p h d", h=HL),
            bvb.rearrange("p (h d) -> p h d", h=HL))

    # ---- attention --------------------------------------------------
    _tail = [None]

    def _flush_tail():
        if _tail[0] is not None:
            _tail[0]()
            _tail[0] = None

    def attention_phase(t, qc, gen):
        """Head pair (2t, 2t+1), query chunk qc: packed scores, exp,
        PV with ones-column; interleaves one filler group per kt."""
        qs = qc * 512
        oaA = psp.tile([DK + 1, 512], F32, tag="oaa", name=f"oaA{t}{qc}")
        oaB = psp.tile([DK + 1, 512], F32, tag="oab", name=f"oaB{t}{qc}")

        def s_exp(kt):
            scA = psp.tile([128, 512], F32, tag=f"s{kt % 2}a")
            scB = psp.tile([128, 512], F32, tag=f"s{kt % 2}b")
            nc.tensor.matmul(scA, KT_sb[0:64, t, kt * 128:(kt + 1) * 128],
                             QT_sb[0:64, t, qs:qs + 512], start=True, stop=True)
            nc.tensor.matmul(scB, KT_sb[64:128, t, kt * 128:(kt + 1) * 128],
                             QT_sb[64:128, t, qs:qs + 512], start=True, stop=True)
            ptA = ptp.tile([128, 512], BF16, tag="pt")
            ptB = ptp.tile([128, 512], BF16, tag="pt")
            nc.scalar.activation(ptA, scA, AF.Exp, scale=float(SCALE))
            nc.scalar.activation(ptB, scB, AF.Exp, scale=float(SCALE))
            return ptA, ptB

        pts = {0: s_exp(0), 1: s_exp(1)}
        _flush_tail()
        for kt in range(NKT):
            if kt + 2 < NKT:
                pts[kt + 2] = s_exp(kt + 2)
            ptA, ptB = pts.pop(kt)
            nc.tensor.matmul(oaA, VA_sb[:, kt, 2 * t, :], ptA,
                             start=(kt == 0), stop=(kt == NKT - 1))
            nc.tensor.matmul(oaB, VA_sb[:, kt, 2 * t + 1, :], ptB,
                             start=(kt == 0), stop=(kt == NKT - 1))
            g = next(gen, None)
            if g is not None:
                g()

        def tail(oaA=oaA, oaB=oaB, t=t, qs=qs):
            for row, oa in ((0, oaA), (64, oaB)):
                rc = misc.tile([1, 512], F32, tag="rc")
                nc.vector.reciprocal(rc, oa[DK:DK + 1, :])
                rcd = dramp.tile([1, 512], F32, tag="rcd")
                nc.gpsimd.dma_start(out=rcd, in_=rc)
                bc = misc.tile([DK, 512], F32, tag="bc")
                nc.gpsimd.dma_start(out=bc, in_=_bcast(rcd[0, :], parts=DK))
                nc.vector.tensor_mul(
                    OT_sb[row:row + DK, t, qs:qs + 512], oa[0:DK, :], bc)

        _tail[0] = tail

    # ---- schedule ---------------------------------------------------
    # pre-phase: Q t0, then (K t0 pc / V kt) interleaved for pc 0-1
    q_group(0, 0)
    q_group(0, 1)
    for pc in range(2):
        k_group(0, pc)
        for kt in range(4 * pc, 4 * pc + 4):
            v_group(kt)

    def filler_gen():
        yield lambda: k_group(0, 2)
        for kt in range(8, 12):
            yield lambda kt=kt: v_group(kt)
        yield lambda: k_group(0, 3)
        for kt in range(12, 16):
            yield lambda kt=kt: v_group(kt)
        for ot in range(1, NTL):
            for qc in range(2):
                yield lambda ot=ot, qc=qc: q_group(ot, qc)
            for pc in range(NPC):
                yield lambda ot=ot, pc=pc: k_group(ot, pc)

    gen = filler_gen()

    # internal DRAM for the Y-half exchange
    agin = [dramp.tile([128, 2, 512], BF16, tag=f"agin{i}", name=f"agin{i}")
            for i in range(2)]
    agout = [dramp.tile([8, 128, 2, 512], BF16, tag=f"agout{i}",
                        name=f"agout{i}", addr_space="Shared")
             for i in range(2)]

    for t in range(NTL):
        for qc in range(2):
            attention_phase(t, qc, gen)
        if t == 1 or t == 3:
            h = t // 2
            _flush_tail()
            nc.gpsimd.dma_start(out=agin[h],
                                in_=OT_sb[:, 2 * h:2 * h + 2, QH:NQ])
            nc.gpsimd.collective_compute(
                "AllGather", OP.bypass,
                replica_groups=[[0, 1, 2, 3, 4, 5, 6, 7]],
                ins=[agin[h].opt()], outs=[agout[h].opt()])
    _flush_tail()
    for g in gen:
        g()

    # fetch partner's Y-halves (dynamic block index from host)
    preg = nc.gpsimd.alloc_register(f"peer_reg_{nc.next_id()}")
    nc.gpsimd.reg_load(preg, psb[0:1, 0:1])
    rv = nc.gpsimd.snap(preg, donate=True, min_val=0, max_val=7)
    for h in range(2):
        nc.gpsimd.dma_start(out=YR_sb[:, 2 * h:2 * h + 2, :],
                            in_=agout[h][bass.ts(rv, 1), :, :, :])

    # keep the PE clock gate hot across the AG wait
    dummy_mm(48)

    # ---- o_proj (full contraction) + LayerNorm ----------------------
    def layer_norm(qt):
        row = yo[:, qt, :]
        stats = misc.tile([128, 2, 6], F32, tag="stats")
        row2 = row.rearrange("p (s n) -> p s n", s=2)
        for s in range(2):
            nc.vector.bn_stats(stats[:, s, :], row2[:, s, :])
        mv = misc.tile([128, 2], F32, tag="mv")
        nc.vector.bn_aggr(mv, stats)
        std = misc.tile([128, 1], F32, tag="std")
        nc.scalar.activation(std, mv[:, 1:2], AF.Sqrt, bias=eps_sb)
        rstd = misc.tile([128, 1], F32, tag="rstd")
        nc.vector.reciprocal(rstd, std)
        nc.vector.tensor_scalar(row, row, mv[:, 0:1], rstd,
                                OP.subtract, OP.mult)
        nc.vector.tensor_mul(row, row, lnwb)
        nc.vector.tensor_add(row, row, lnbb)
        nc.sync.dma_start(out=out[qt * 128:(qt + 1) * 128, :], in_=row)

    for qt in range(NQ // 256):
        for dc in range(2):
            ps = psp.tile([128, 512], F32, tag=f"p{dc % 2}", name=f"o{qt}{dc}")
            for tl in range(NTL):
                nc.tensor.matmul(ps, OT_sb[:, tl, qt * 128:(qt + 1) * 128],
                                 wo[:, 0, tl, dc * 512:(dc + 1) * 512],
                                 start=(tl == 0), stop=False)
            for tl in range(NTL):
                nc.tensor.matmul(ps, YR_sb[:, tl, qt * 128:(qt + 1) * 128],
                                 wo[:, 1, tl, dc * 512:(dc + 1) * 512],
                                 start=False, stop=(tl == NTL - 1))
            nc.vector.tensor_add(yo[:, qt, dc * 512:(dc + 1) * 512], ps,
                                 bob[:, dc * 512:(dc + 1) * 512])
        layer_norm(qt)


# ---------------------------------------------------------------------------
# host side: cached PJRT runner
# ---------------------------------------------------------------------------
_CACHE = {}


class _Runner:
    def __init__(self, nc, n_cores=8, donate=True):
        import jax
        from jax.experimental.shard_map import shard_map
        from jax.sharding import Mesh, PartitionSpec

        from concourse import bass2jax

        bass2jax.install_neuronx_cc_hook()
        self.jax = jax
        self.n_cores = n_cores
        partition_name = (
            nc.partition_id_tensor.name if nc.partition_id_tensor else None)
        in_names, out_names, out_avals = [], [], []
        for alloc in nc.m.functions[0].allocations:
            if not isinstance(alloc, mybir.MemoryLocationSet):
                continue
            name = alloc.memorylocations[0].name
            if alloc.kind == "ExternalInput":
                if name != partition_name:
                    in_names.append(name)
            elif alloc.kind == "ExternalOutput":
                out_names.append(name)
                out_avals.append(jax.core.ShapedArray(
                    tuple(alloc.tensor_shape), mybir.dt.np(alloc.dtype)))
        self.param_names = in_names
        self.out_names = out_names
        self.out_avals = out_avals
        n_params = len(in_names)
        all_in = list(in_names) + list(out_names)
        if partition_name is not None:
            all_in.append(partition_name)

        def _body(*args):
            operands = list(args)
            if partition_name is not None:
                operands.append(bass2jax.partition_id_tensor())
            return tuple(bass2jax._bass_exec_p.bind(
                *operands,
                out_avals=tuple(out_avals),
                in_names=tuple(all_in),
                out_names=tuple(out_names),
                lowering_input_output_aliases=(),
                sim_require_finite=True,
                sim_require_nnan=True,
                nc=nc,
            ))

        devices = jax.devices()[:n_cores]
        self.mesh = Mesh(np.asarray(devices), ("core",))
        donate_idx = (
            tuple(range(n_params, n_params + len(out_names))) if donate else ())
        in_specs = (PartitionSpec("core"),) * (n_params + len(out_names))
        out_specs = (PartitionSpec("core"),) * len(out_names)
        self.fn = jax.jit(
            shard_map(_body, mesh=self.mesh, in_specs=in_specs,
                      out_specs=out_specs, check_rep=False),
            donate_argnums=donate_idx, keep_unused=True)

    def concat_inputs(self, in_maps):
        return [
            np.concatenate([np.asarray(m[n]) for m in in_maps], axis=0)
            for n in self.param_names
        ]

    def zeros(self):
        return [
            np.zeros((self.n_cores * a.shape[0], *a.shape[1:]), a.dtype)
            for a in self.out_avals
        ]

    def run_concat(self, concat_in, zeros=None):
        if zeros is None:
            zeros = self.zeros()
        outs = self.fn(*concat_in, *zeros)
        self.jax.block_until_ready(outs)
        return outs

    def __call__(self, in_maps):
        outs = self.run_concat(self.concat_inputs(in_maps))
        res = []
        for c in range(self.n_cores):
            res.append({
                name: np.asarray(outs[i]).reshape(
                    self.n_cores, *self.out_avals[i].shape)[c]
                for i, name in enumerate(self.out_names)
            })
        return res


def _get_runner(repeat=1, donate=True):
    key = (repeat, donate)
    if key not in _CACHE:
        _CACHE[key] = _Runner(_build(repeat), donate=donate)
    return _CACHE[key]


BF_NP = mybir.dt.np(BF16)


def _sbuf_image(mat2d, dtype=BF_NP):
    """[D, n] -> [128, D//128, n] SBUF image (partition-major)."""
    d, n = mat2d.shape
    return np.ascontiguousarray(
        mat2d.reshape(d // 128, 128, n).transpose(1, 0, 2).astype(dtype))


def make_in_maps(q, C, Wq, bq, Wk, bk, Wv, bv, Wo, bo, ln_w, ln_b):
    f32 = lambda x: np.ascontiguousarray(np.asarray(x, dtype=np.float32))
    q, C = f32(q), f32(C)
    Wq, Wk, Wv, Wo = f32(Wq), f32(Wk), f32(Wv), f32(Wo)
    bq, bk, bv, bo, ln_w, ln_b = map(f32, (bq, bk, bv, bo, ln_w, ln_b))
    WoTf = Wo.T  # [in, out]

    CTs = []
    for b in range(B):
        img = _sbuf_image(np.ascontiguousarray(C[b].T))   # [128, NT, P]
        CTs.append(np.ascontiguousarray(
            img.reshape(128, NT, NPC, 512).transpose(0, 2, 1, 3)))

    halves = {}
    for g in range(2):
        sl = slice(g * DL, (g + 1) * DL)
        halves[g] = dict(
            WqT=_sbuf_image(np.ascontiguousarray(Wq[sl, :].T)),
            WkT=_sbuf_image(np.ascontiguousarray(Wk[sl, :].T)),
            WvT=_sbuf_image(np.ascontiguousarray(Wv[sl, :].T)),
            WoTh=_sbuf_image(np.ascontiguousarray(WoTf[sl, :])),
            bq=np.ascontiguousarray(bq[sl]),
            bk=np.ascontiguousarray(bk[sl]),
            bv=np.ascontiguousarray(bv[sl]),
        )

    in_maps = []
    for c in range(8):
        b, g = c // 2, c % 2
        hv = halves[g]
        q_perm = np.concatenate(
            [q[b, g * QH:(g + 1) * QH, :], q[b, (1 - g) * QH:(2 - g) * QH, :]],
            axis=0)
        qTs = _sbuf_image(np.ascontiguousarray(q_perm.T))
        WoT = np.ascontiguousarray(
            np.stack([hv["WoTh"], halves[1 - g]["WoTh"]], axis=1))
        in_maps.append({
            "qT": qTs, "CT": CTs[b],
            "WqT": hv["WqT"], "WkT": hv["WkT"], "WvT": hv["WvT"],
            "WoT": WoT,
            "bq": hv["bq"], "bk": hv["bk"], "bv": hv["bv"], "bo": bo,
            "lnw": ln_w, "lnb": ln_b,
            "peer": np.array([[c ^ 1]], np.uint32),
        })
    return in_maps


def kernel(q, C, Wq, bq, Wk, bk, Wv, bv, Wo, bo, ln_w, ln_b):
    in_maps = make_in_maps(q, C, Wq, bq, Wk, bk, Wv, bv, Wo, bo, ln_w, ln_b)
    res = _get_runner(1)(in_maps)
    out = np.empty((B, NQ, D), dtype=np.float32)
    for c in range(8):
        b, g = c // 2, c % 2
        out[b, g * QH:(g + 1) * QH, :] = res[c]["out"]
    return out
